# revision 1
# baseline (speedup 1.0000x reference)
"""MultiHeadAttention Trainium2 kernel — bf16 projections + fp8 DoubleRow
scores + bf16 attention/output path.

Core c: batch b=c//2, heads [(c%2)*8, (c%2)*8+8) (512-wide D_MODEL slice).
Host sums the two partial output projections per batch and adds bo.

Cost-model-driven design (TimelineSim charges out_free_size c/row; bf16 =
1 c/row at any size; fp8 DoubleRow = 0.5 c/row and contracts 2x128 rows
per instruction):
  - Q/K/V projections bf16 (x1/x2/w* sent as bf16 from host; wq/wk columns
    host-permuted so psum partitions land in the scrambled
    (hmod4*32 + dmod32) layout that scores DoubleRow wants); the bias-add
    writes qt8/kt8 as fp8e4 — the only fp8 quantization on the path.
  - qt8/kt8 [128, hslot, dj, S] fp8; scores per head = ONE DoubleRow
    matmul [32,2,128]x[32,2,512] -> psum [128 k, 512 q] (256 cycles,
    half the bf16 cost).
  - exp (ACT) psum -> ex bf16 [128, 2, 512] per (pair, kc): the ACT
    engine is the global bottleneck (~266 us busy).
  - attn@V: stationary ex [128k, 128q] bf16, moving V [128k, 64] bf16 ->
    psum outp [128 q, 2 par, 4 qc, 64] (one bank per pair, 64 c/row
    instead of the naive 512); denominators via 1-column ones matmuls
    into a shared den bank (per-pair accumulation groups).
  - normalize: DVE reciprocal + tensor_scalar_mul -> ao bf16; xbar
    DMA-transpose [128q, 128(par,d)] -> aot [d, q] bf16.
  - output projection bf16 from aot x wo.
Emission is one flat software-pipelined stream over 256 (qb,pair,kc)
ticks: per tick: exp(t) | loads | one EDF-scheduled heavy fill (qt/kt/v
chunk or output-projection block, deadline = first scores emission that
reads it) | deferred attn@V from a bounded backlog | scores(t+2) last so
fills never sit behind the exp-gated score matmul in the in-order PE
queue. attn@V release is gated on its v-chunk having been emitted
(emission order defines dependency order). PSUM: scores 2x2 banks +
outp 2x1 + den 1 + fill scratch 1 = 8 banks.
"""

import sys

sys.path.insert(0, "/opt/trn_rl_repo")

from contextlib import ExitStack

import numpy as np
import concourse.bass as bass
import concourse.tile as tile
from concourse import bacc, mybir
from concourse.bass_utils import run_bass_kernel_spmd

B, S, D_IN, D_MODEL, H = 4, 2048, 1024, 1024, 16
DH = 64
HPC = 8
DS = 512
F32 = mybir.dt.float32
BF16 = mybir.dt.bfloat16
FP8 = mybir.dt.float8e4
DRow = mybir.MatmulPerfMode.DoubleRow
Exp = mybir.ActivationFunctionType.Exp

NKC = D_IN // 128  # 8
NSC = S // 128  # 16
QB = 512
SCALE = 1.0 / np.sqrt(DH)
D_LAG = 27  # max attn@V backlog (ticks) behind the exp stream
D_MIN = 2  # min backlog: released attn@V must be stale so it never gates PE


def _col_perm():
    """Permuted D-column order for wq/wk so proj psum partitions match the
    scores-DoubleRow layout: chunk c=(hslot,dj), partition p ->
    col = 64*(p//32 + 4*hslot) + 32*dj + p%32."""
    perm = np.empty(DS, np.int64)
    for c in range(4):
        hslot, dj = c // 2, c % 2
        for p in range(128):
            perm[c * 128 + p] = 64 * (p // 32 + 4 * hslot) + 32 * dj + p % 32
    return perm


def _kernel_body(nc, tc, aps):
    x1, x2, wq, wk, wv, wo, bq, bk, bv, out = aps

    with ExitStack() as ctx:
        pers = ctx.enter_context(tc.tile_pool(name="pers", bufs=1))

        wq_sb = pers.tile([128, NKC, DS], BF16)  # host-permuted d columns
        wk_sb = pers.tile([128, NKC, DS], BF16)
        wv_sb = pers.tile([128, NKC, DS], BF16)
        wo_sb = pers.tile([128, 4, D_MODEL], BF16)
        qt8 = pers.tile([128, 2, 2, S], FP8)  # [p, hslot, dj, q]
        kt8 = pers.tile([128, 2, 2, S], FP8)
        v_sb = pers.tile([128, NSC, HPC, DH], BF16)
        aot = pers.tile([128, 4, S], BF16)
        bq_sb = pers.tile([128, 4], F32)
        bk_sb = pers.tile([128, 4], F32)
        bv_bc = pers.tile([128, DS], F32)
        ones = pers.tile([128, 1], BF16)

        px1 = ctx.enter_context(tc.tile_pool(name="px1", bufs=2))
        px2 = ctx.enter_context(tc.tile_pool(name="px2", bufs=4))
        xts = {}

        nc.gpsimd.memset(ones[:, :], 1.0)
        nc.gpsimd.dma_start(
            out=bq_sb, in_=bq.rearrange("(c p) o -> p (c o)", p=128)
        )
        nc.gpsimd.dma_start(
            out=bk_sb, in_=bk.rearrange("(c p) o -> p (c o)", p=128)
        )
        nc.gpsimd.dma_start(
            out=bv_bc, in_=bv.rearrange("s o -> o s").to_broadcast([128, DS])
        )

        def ld_x8(which, sq, eng):
            x, pool = (x1, px1) if which == 1 else (x2, px2)
            t = pool.tile([128, NKC, QB], BF16, tag="x8", name=f"x8_{which}_{sq}")
            eng.dma_start(
                out=t,
                in_=x.rearrange("(c p) s -> p c s", p=128)[
                    :, :, sq * QB : (sq + 1) * QB
                ],
            )
            xts[(which, sq)] = t

        # bf16 projection chunk c of quarter sq -> fp8 qt8/kt8 (columns are
        # host-permuted so psum partitions land in the scores-DoubleRow
        # layout; bf16 operands keep the scores-input quantization as the
        # only fp8 error on this path)
        def qk_chunk(psp, which, sq, c):
            w_sb, dst, b_sb = (
                (wq_sb, qt8, bq_sb) if which == 1 else (wk_sb, kt8, bk_sb)
            )
            xt = xts[(which, sq)]
            ps = psp.tile([128, QB], F32, tag="pp", name="qkp")
            for kc in range(NKC):
                nc.tensor.matmul(
                    ps,
                    w_sb[:, kc, c * 128 : (c + 1) * 128],
                    xt[:, kc, :],
                    start=(kc == 0),
                    stop=(kc == NKC - 1),
                )
            nc.vector.tensor_scalar_add(
                dst[:, c // 2, c % 2, sq * QB : (sq + 1) * QB],
                ps,
                b_sb[:, c : c + 1],
            )

        def v_chunk(psp, sq, sc):
            xt = xts[(2, sq)]
            ps = psp.tile([128, QB], F32, tag="pp", name="vp")
            for kc in range(NKC):
                nc.tensor.matmul(
                    ps,
                    xt[:, kc, sc * 128 : (sc + 1) * 128],
                    wv_sb[:, kc, :],
                    start=(kc == 0),
                    stop=(kc == NKC - 1),
                )
            nc.vector.tensor_add(
                v_sb[:, sq * 4 + sc, :, :],
                ps.rearrange("p (h d) -> p h d", h=HPC),
                bv_bc.rearrange("p (h d) -> p h d", h=HPC),
            )

        # ---- load streams ----
        # DMA transfers serialize on the modeled DMA engines, so order IS the
        # schedule. Scalar carries only the two loads the first scores need
        # (each DMA dispatch costs ~667ns on the ACT sequencer before exp 0).
        # Weight tiles are split so the startup chunks (dcols 0:256) unblock
        # after half a load.
        dma_ready = {}
        _dma_clk = [2000.0]

        def _track(name, bytes_pp, emit):
            emit()
            _dma_clk[0] += bytes_pp * 0.3855 + 200.0
            dma_ready[name] = _dma_clk[0]

        wq_r = wq.rearrange("(c p) d -> p c d", p=128)
        wk_r = wk.rearrange("(c p) d -> p c d", p=128)
        _track(
            "wk8h",
            4096,
            lambda: nc.scalar.dma_start(
                out=wk_sb[:, :, 0:256], in_=wk_r[:, :, 0:256]
            ),
        )
        _track("x2q0", 8192, lambda: ld_x8(2, 0, nc.scalar))
        _track(
            "wq8h",
            4096,
            lambda: nc.sync.dma_start(
                out=wq_sb[:, :, 0:256], in_=wq_r[:, :, 0:256]
            ),
        )
        _track("x1q0", 8192, lambda: ld_x8(1, 0, nc.sync))
        _track(
            "wv",
            8192,
            lambda: nc.sync.dma_start(
                out=wv_sb, in_=wv.rearrange("(c p) o -> p c o", p=128)
            ),
        )
        _track("x2q1", 8192, lambda: ld_x8(2, 1, nc.sync))
        _track("x2q2", 8192, lambda: ld_x8(2, 2, nc.sync))
        _track(
            "wk8r",
            4096,
            lambda: nc.sync.dma_start(
                out=wk_sb[:, :, 256:512], in_=wk_r[:, :, 256:512]
            ),
        )
        _track(
            "wq8r",
            4096,
            lambda: nc.sync.dma_start(
                out=wq_sb[:, :, 256:512], in_=wq_r[:, :, 256:512]
            ),
        )
        _track("x2q3", 8192, lambda: ld_x8(2, 3, nc.sync))
        for s in range(4):
            dma_ready[f"xbq{s}"] = dma_ready[f"x2q{s}"]

        # startup projection chunks in their own psum pool (closed before the
        # attention pools take the banks): scores of pair 0 read dj 0 AND 1
        # of hslot 0, i.e. chunks 0 and 1 of qt8/kt8 quarter 0. A chain of
        # dummy matmuls on memset tiles first: they run while the input DMAs
        # are in flight and ramp the PE clock out of its cold p-state so the
        # real chunks run at full speed.
        warm = pers.tile([128, 512], BF16)
        nc.gpsimd.memset(warm[:, :], 0.0)
        with tc.tile_pool(name="psA", bufs=4, space="PSUM") as psA:
            wps = psA.tile([128, 512], F32, tag="pp", name="warmp")
            for i in range(8):
                nc.tensor.matmul(
                    wps[0:1, :],
                    ones[:, 0:1],
                    warm[:, :],
                    start=(i == 0),
                    stop=(i == 7),
                )
            qk_chunk(psA, 1, 0, 0)
            qk_chunk(psA, 2, 0, 0)
            qk_chunk(psA, 1, 0, 1)
            qk_chunk(psA, 2, 0, 1)

        # ---- attention pools ----
        attn_ctx = ctx.enter_context(ExitStack())
        psc = attn_ctx.enter_context(tc.tile_pool(name="psc", bufs=2, space="PSUM"))
        pso = attn_ctx.enter_context(tc.tile_pool(name="pso", bufs=2, space="PSUM"))
        pdn = attn_ctx.enter_context(tc.tile_pool(name="pdn", bufs=1, space="PSUM"))
        psp = attn_ctx.enter_context(tc.tile_pool(name="psp", bufs=1, space="PSUM"))
        pex = attn_ctx.enter_context(
            tc.tile_pool(name="pex", bufs=D_LAG + 3)
        )
        pao = attn_ctx.enter_context(tc.tile_pool(name="pao", bufs=2))
        prd = attn_ctx.enter_context(tc.tile_pool(name="prd", bufs=2))
        pot = attn_ctx.enter_context(tc.tile_pool(name="pot", bufs=2))

        def oproj_mb(mb):
            ot = pot.tile([128, D_MODEL], F32, tag="ot", name="oti")
            for nt in range(2):
                ps = psp.tile([128, 512], F32, tag="pp", name="opp")
                for kc in range(4):
                    nc.tensor.matmul(
                        ps,
                        aot[:, kc, mb * 128 : (mb + 1) * 128],
                        wo_sb[:, kc, nt * 512 : (nt + 1) * 512],
                        start=(kc == 0),
                        stop=(kc == 3),
                    )
                nc.vector.tensor_copy(ot[:, nt * 512 : (nt + 1) * 512], ps)
            nc.sync.dma_start(out=out[mb * 128 : (mb + 1) * 128, :], in_=ot)

        # ---- filler schedule: tick -> thunks ----
        # chunk c=(hslot,dj) of a quarter serves pairs 2*(c//2), 2*(c//2)+1.
        from collections import defaultdict

        CV, CQK, COP = 1707, 1707, 1707
        T0, TICK = 15500.0, 1038.0
        MAX_BACKLOG = D_LAG
        AGE = 10

        def r2t(ns):
            return max(0, int((ns - T0) / TICK) + 1)

        # jobs: (deadline_tick, ready_tick, pe_cost, thunk). One heavy job
        # per tick, earliest-deadline-first among ready jobs: PE passes the
        # exp-gated score matmul of the previous tick and must fit a tick's
        # job into ~2 ACT periods, so a single <=1.7us job never stalls the
        # exp stream once the PE clock is ramped.
        jobs = []  # (deadline, ready_tick, pe_cost, thunk, tag)
        for s in range(1, 4):
            for c in range(2):
                jobs.append(
                    (4 * s - 3 + c, r2t(dma_ready[f"x2q{s}"]), CQK,
                     lambda s=s, c=c: qk_chunk(psp, 2, s, c), None)
                )
        # pair 2 (tick 32+4s) reads BOTH dj chunks (c2 and c3) of hslot 1
        for s in range(4):
            rd = r2t(max(dma_ready[f"x2q{s}"], dma_ready["wk8r"]))
            jobs.append(
                (29 + 4 * s, rd, CQK, lambda s=s: qk_chunk(psp, 2, s, 2), None)
            )
            jobs.append(
                (30 + 4 * s, rd, CQK, lambda s=s: qk_chunk(psp, 2, s, 3), None)
            )
        for c in (2, 3):
            jobs.append(
                (27 + c, r2t(max(dma_ready["x1q0"], dma_ready["wq8r"])), CQK,
                 lambda c=c: qk_chunk(psp, 1, 0, c), None)
            )
        for s in range(4):
            rv = r2t(max(dma_ready[f"xbq{s}"], dma_ready["wv"]))
            for sc in range(4):
                jobs.append(
                    (4 * s + sc + AGE - 1, rv, CV,
                     lambda s=s, sc=sc: v_chunk(psp, s, sc), ("v", 4 * s + sc))
                )
        for sq in range(1, 4):
            for c in range(4):
                jobs.append(
                    (64 * sq - 3 + c if c < 2 else 64 * sq + 27 + c,
                     64 * (sq - 1) + 6, CQK,
                     lambda sq=sq, c=c: qk_chunk(psp, 1, sq, c), None)
                )
        # ready: the source qb's last transpose is emitted when its final
        # attn@V leaves the backlog (qb end + AGE), not at qb end itself
        for mb in range(12):
            jobs.append(
                (1000 + mb, 64 * (mb // 4 + 1) + AGE + 2, COP,
                 lambda mb=mb: oproj_mb(mb), None)
            )
        jobs.sort(key=lambda j: (j[0], j[1]))
        # Precompute the EDF tick assignment; kt/qt jobs must land by their
        # deadline (scores would otherwise read uninitialized sbuf), v jobs
        # gate attn@V release below.
        v_done_tick = {}
        _pending = list(range(len(jobs)))
        assigned = {}
        for t in range(256):
            pick = None
            for idx in _pending:
                # a kt/qt chunk at its deadline MUST be emitted now (a later
                # emission would be a read-before-write race with the score
                # matmuls); otherwise take the earliest-deadline ready job
                if jobs[idx][0] <= t or jobs[idx][1] <= t:
                    pick = idx
                    break
            if pick is not None:
                assigned[t] = pick
                _pending.remove(pick)
                tag = jobs[pick][4]
                if tag and tag[0] == "v":
                    v_done_tick[tag[1]] = t
        assert not _pending, f"{len(_pending)} jobs unassigned"
        assert len(v_done_tick) == 16
        # zero-cost emissions (loads) at fixed ticks
        fill = defaultdict(list)
        for sq in range(1, 4):
            fill[64 * (sq - 1) + 2].append(lambda sq=sq: ld_x8(1, sq, nc.sync))
        fill[20].append(
            lambda: nc.sync.dma_start(
                out=wo_sb, in_=wo.rearrange("(c p) o -> p c o", p=128)
            )
        )

        # ---- flat pipelined attention stream ----
        TICKS = [
            (qb, pair, kc)
            for qb in range(4)
            for pair in range(4)
            for kc in range(NSC)
        ]
        sc_tiles = {}
        ex_tiles = {}
        state = {}

        def emit_sc(t):
            qb, pair, kc = TICKS[t]
            q0 = qb * QB
            scp = psc.tile([128, 2, QB], F32, tag="sc", name="scp")
            for par in range(2):
                h = 2 * pair + par
                hb = 32 * (h % 4)
                nc.tensor.matmul(
                    scp[:, par, :],
                    kt8[hb : hb + 32, h // 4, :, kc * 128 : (kc + 1) * 128],
                    qt8[hb : hb + 32, h // 4, :, q0 : q0 + QB],
                    start=True,
                    stop=True,
                    perf_mode=DRow,
                    tile_position=(hb, 0),
                )
            sc_tiles[t] = scp

        def emit_exp(t):
            expt = pex.tile([128, 2, QB], BF16, tag="ex", name="ext")
            nc.scalar.activation(expt, sc_tiles.pop(t), Exp, scale=float(SCALE))
            ex_tiles[t] = expt

        def emit_av(t):
            qb, pair, kc = TICKS[t]
            expt = ex_tiles.pop(t)
            if kc == 0:
                state[("outp", pair % 2)] = pso.tile(
                    [128, 2, 4, DH], F32, tag="acc", name="outp"
                )
                if pair == 0:
                    state["den"] = pdn.tile([128, 32], F32, tag="dn", name="den")
                    state["rden"] = prd.tile([128, 32], F32, tag="rd", name="rden")
            outp = state[("outp", pair % 2)]
            den = state["den"]
            for par in range(2):
                h = 2 * pair + par
                for qc in range(4):
                    exs = expt[:, par, qc * 128 : (qc + 1) * 128]
                    first = kc == 0 and par == 0 and qc == 0
                    last = kc == NSC - 1 and par == 1 and qc == 3
                    nc.tensor.matmul(
                        outp[:, par, qc, :],
                        exs,
                        v_sb[:, kc, h, :],
                        start=first,
                        stop=last,
                    )
                    di = pair * 8 + par * 4 + qc
                    nc.tensor.matmul(
                        den[:, di : di + 1],
                        exs,
                        ones[:, :],
                        start=first,
                        stop=last,
                    )
            if kc == NSC - 1:
                q0 = qb * QB
                rden = state["rden"]
                ao = pao.tile([128, 4, 2, DH], BF16, tag="ao", name="ao")
                last = qb == 3 and pair == 3
                for qc in range(4):
                    for par in range(2):
                        di = pair * 8 + par * 4 + qc
                        nc.vector.reciprocal(
                            rden[:, di : di + 1], den[:, di : di + 1]
                        )
                        nc.vector.tensor_scalar_mul(
                            ao[:, qc, par, :],
                            outp[:, par, qc, :],
                            rden[:, di : di + 1],
                        )
                    # last pair of the kernel: alternate transpose queues and
                    # let the tail oproj start per-qc
                    eng = nc.scalar if (last and qc % 2 == 1) else nc.sync
                    eng.dma_start_transpose(
                        aot[:, pair, q0 + qc * 128 : q0 + (qc + 1) * 128],
                        ao[:, qc, :, :],
                    )

        # Greedy emission: track modeled PE/ACT clocks; defer attn@V work
        # (bounded backlog) whenever PE is at risk of starving the ACT
        # stream, and drain it in ACT-bound stretches.
        C_SC, C_EXP, C_AV = 213.0, 1038.0, 220.0
        # ACT's first exp lands ~T0 after PE starts (startup DMA chain +
        # cold-clock projection chunks); bias the modeled ACT clock so the
        # greedy defers attn@V work during the early PE-heavy stretch.
        pe_t, act_t = 0.0, T0
        sc_done = {}
        backlog = []

        emit_sc(0)
        sc_done[0] = pe_t = C_SC
        emit_sc(1)
        sc_done[1] = pe_t = pe_t + C_SC
        for t in range(256):
            act_t = max(act_t, sc_done[t] + 100.0) + C_EXP
            emit_exp(t)
            backlog.append(t)
            for f in fill[t]:
                f()
            job_done = False
            if t in assigned:
                dl, rd, cost, th, tag = jobs[assigned[t]]
                th()
                pe_t += cost
                job_done = True
            # attn@V after the job: these small matmuls overlap the job's
            # psum-drain latency so back-to-back fills don't bubble PE.
            # Age-based release (rather than a modeled PE clock, which
            # drifts): every attn@V runs AGE ticks behind its exp, so the
            # ex pool never starves the exp stream and the backlog drains
            # deterministically. Never release an attn@V whose v chunk
            # hasn't been emitted yet.
            while backlog and (
                len(backlog) > MAX_BACKLOG
                or (len(backlog) > D_MIN and t - backlog[0] >= AGE)
            ):
                qbu, pairu, kcu = TICKS[backlog[0]]
                if qbu == 0 and v_done_tick[kcu] > t:
                    break
                emit_av(backlog.pop(0))
                pe_t += C_AV
            # exp-gated score matmul last, so jobs/attn@V never sit behind
            # the gate in the PE queue
            if t + 2 < 256:
                emit_sc(t + 2)
                pe_t += C_SC
                sc_done[t + 2] = pe_t
        for u in backlog:
            emit_av(u)

        attn_ctx.close()

        # ---- tail: last output-projection blocks ----
        with tc.tile_pool(name="psD", bufs=4, space="PSUM") as psD, tc.tile_pool(
            name="potD", bufs=2
        ) as potD:
            for mb in range(12, 16):
                ot = potD.tile([128, D_MODEL], F32, tag="ot", name="otd")
                for nt in range(2):
                    ps = psD.tile([128, 512], F32, tag="pf", name="opd")
                    for kc in range(4):
                        nc.tensor.matmul(
                            ps,
                            aot[:, kc, mb * 128 : (mb + 1) * 128],
                            wo_sb[:, kc, nt * 512 : (nt + 1) * 512],
                            start=(kc == 0),
                            stop=(kc == 3),
                        )
                    nc.vector.tensor_copy(ot[:, nt * 512 : (nt + 1) * 512], ps)
                nc.sync.dma_start(out=out[mb * 128 : (mb + 1) * 128, :], in_=ot)


_NC_CACHE = []


def _build():
    if _NC_CACHE:
        return _NC_CACHE[0]
    nc = bacc.Bacc(None, target_bir_lowering=False, debug=False)
    x1 = nc.dram_tensor("x1", [D_IN, S], BF16, kind="ExternalInput")
    x2 = nc.dram_tensor("x2", [D_IN, S], BF16, kind="ExternalInput")
    wq = nc.dram_tensor("wq", [D_IN, DS], BF16, kind="ExternalInput")
    wk = nc.dram_tensor("wk", [D_IN, DS], BF16, kind="ExternalInput")
    wv = nc.dram_tensor("wv", [D_IN, DS], BF16, kind="ExternalInput")
    wo = nc.dram_tensor("wo", [DS, D_MODEL], BF16, kind="ExternalInput")
    bq = nc.dram_tensor("bq", [DS, 1], F32, kind="ExternalInput")
    bk = nc.dram_tensor("bk", [DS, 1], F32, kind="ExternalInput")
    bv = nc.dram_tensor("bv", [DS, 1], F32, kind="ExternalInput")
    out = nc.dram_tensor("out", [S, D_MODEL], F32, kind="ExternalOutput")
    with tile.TileContext(nc) as tc:
        _kernel_body(
            nc,
            tc,
            aps=(
                x1[:, :],
                x2[:, :],
                wq[:, :],
                wk[:, :],
                wv[:, :],
                wo[:, :],
                bq[:, :],
                bk[:, :],
                bv[:, :],
                out[:, :],
            ),
        )
    nc.compile()
    _NC_CACHE.append(nc)
    return nc


def _run(inputs, trace=False, **kw):
    import ml_dtypes

    nc = _build()
    F8 = ml_dtypes.float8_e4m3fn
    BF = ml_dtypes.bfloat16
    f32 = lambda a: np.ascontiguousarray(np.asarray(a, dtype=np.float32))
    perm = _col_perm()
    X1, X2 = (
        np.asarray(inputs["X1"], np.float32),
        np.asarray(inputs["X2"], np.float32),
    )
    Wq, Wk = np.asarray(inputs["Wq"], np.float32), np.asarray(
        inputs["Wk"], np.float32
    )
    Wv, Wo = np.asarray(inputs["Wv"], np.float32), np.asarray(
        inputs["Wo"], np.float32
    )
    bqf, bkf = (
        np.asarray(inputs["bq"], np.float32),
        np.asarray(inputs["bk"], np.float32),
    )
    in_maps = []
    for c in range(8):
        b, hf = c // 2, c % 2
        sl = slice(hf * DS, (hf + 1) * DS)
        wq_s, wk_s = Wq[:, sl][:, perm], Wk[:, sl][:, perm]
        in_maps.append(
            {
                "x1": np.ascontiguousarray(X1[b].T).astype(BF),
                "x2": np.ascontiguousarray(X2[b].T).astype(BF),
                "wq": np.ascontiguousarray(wq_s).astype(BF),
                "wk": np.ascontiguousarray(wk_s).astype(BF),
                "wv": np.ascontiguousarray(Wv[:, sl]).astype(BF),
                "wo": np.ascontiguousarray(Wo[sl, :]).astype(BF),
                "bq": np.ascontiguousarray(bqf[sl][perm]).reshape(DS, 1),
                "bk": np.ascontiguousarray(bkf[sl][perm]).reshape(DS, 1),
                "bv": f32(inputs["bv"])[sl].reshape(DS, 1),
            }
        )
    res = run_bass_kernel_spmd(nc, in_maps, list(range(8)), trace=trace, **kw)
    parts = [res.results[c]["out"] for c in range(8)]
    bo = f32(inputs["bo"])
    full = np.stack(
        [parts[2 * b] + parts[2 * b + 1] + bo[None, :] for b in range(B)]
    )
    return full.astype(np.float32), res


def kernel(**inputs):
    out, _ = _run(inputs, trace=False)
    return out



# revision 4
# speedup vs baseline: 1.1491x; 1.1491x over previous
"""MultiHeadAttention Trainium2 kernel — fp8 DoubleRow Q/K projections +
fp8 DoubleRow scores + dual-engine (ACT exact / DVE fast-exp) softmax +
bf16 attention/output path.

Core c: batch b=c//2, heads [(c%2)*8, (c%2)*8+8) (512-wide D_MODEL slice).
Host sums the two partial output projections per batch and adds bo.

Cost-model-driven design (TimelineSim charges matmuls out_free_size c/row;
bf16 = 1 c/row, fp8 DoubleRow = 0.5 c/row contracting 2x128 rows per
instruction; ACT = 0.83 ns/elem, DVE = 1.04 ns/elem from f32 psum):
  - Q/K projections fp8 DoubleRow (x1/x2/wq/wk host-quantized to fp8 in
    [64, 2pair, kc, .] layout; wq/wk columns host-permuted so psum
    partitions land in the scrambled (hmod4*32 + dmod32) layout that
    scores DoubleRow wants); the bias-add+quantize to fp8 qt8/kt8 runs on
    ACT (Identity+bias) or DVE (tensor_scalar) whichever is less loaded.
  - V projection bf16 (from a separate bf16 copy of x2): fp8 V error
    (~2.5%/elem) passes through attention averaging undamped and would
    blow the 2e-2 gate; bf16 V is ~0.2%.
  - scores per head = ONE DoubleRow matmul [32,2,128]x[32,2,512] ->
    psum [128 k, 512 q].
  - exp: split between ACT (exact activation, ~1.04us/tick) and DVE
    (Schraudolph fast-exp: i16 = round(score*16*log2e + 16248.5) written
    into the bf16 ex tile via .bitcast(int16) = piecewise-linear exp with
    ~1.8% rms / 4.2% max per-element error; quota-capped so total output
    error stays well under the 2e-2 gate). Both produce bf16 ex tiles;
    attn@V and den read them identically.
  - attn@V bf16 as before; denominators via 1-column ones matmuls.
  - normalize: one DVE reciprocal [128,8] per pair + ONE broadcast
    tensor_tensor (outp * rden with a stride-0 ap) -> ao bf16; xbar
    DMA-transpose -> aot.
  - output projection bf16; psum halves drained by ACT-Copy or DVE-copy
    (least-loaded), DMA'd to HBM from the gpsimd (Pool) queue -- Pool is
    otherwise idle and its SWDGE descriptor generation is free ACT/DVE
    time.
Emission is one flat software-pipelined stream over 256 (qb,pair,kc)
ticks as in the previous version: per tick: exp(t) on the engine with the
earlier modeled finish | loads | one EDF-scheduled heavy PE fill | deferred
attn@V from a bounded backlog (age-released, v-chunk-gated) | scores(t+2)
last. PSUM: scores 2x2 banks + outp 2x1 + den 1 + fill scratch 1 = 8.
"""

import sys

sys.path.insert(0, "/opt/trn_rl_repo")

from collections import defaultdict
from contextlib import ExitStack

import numpy as np
import concourse.bass as bass
import concourse.tile as tile
from concourse import bacc, mybir
from concourse.bass_utils import run_bass_kernel_spmd

B, S, D_IN, D_MODEL, H = 4, 2048, 1024, 1024, 16
DH = 64
HPC = 8
DS = 512
F32 = mybir.dt.float32
BF16 = mybir.dt.bfloat16
FP8 = mybir.dt.float8e4
I16 = mybir.dt.int16
DRow = mybir.MatmulPerfMode.DoubleRow
Exp = mybir.ActivationFunctionType.Exp
Ident = mybir.ActivationFunctionType.Identity
Copy = mybir.ActivationFunctionType.Copy
Mult = mybir.AluOpType.mult
Add = mybir.AluOpType.add

NKC = D_IN // 128  # 8
NSC = S // 128  # 16
QB = 512
SCALE = 1.0 / np.sqrt(DH)
# Schraudolph fast-exp on DVE: bf16 bits of e^(score*SCALE) ~=
# round(score * 128*SCALE*log2(e) + 127*128 - 7.5); -7.5 centers the
# piecewise-linear sawtooth (1.8% rms / 4.2% max, measured on HW).
AEXP = 16.0 * float(np.log2(np.e))
BEXP = 16256.0 - 7.5
DVE_EXP_MAX = 96  # max fast-exp ticks (error budget dial)

D_MIN = 2  # min backlog: released attn@V must be stale so it never gates PE
AGE = 10  # attn@V runs AGE ticks behind its exp
MAX_BACKLOG = 16
EX_BUFS = 18


def _col_perm():
    """Permuted D-column order for wq/wk so proj psum partitions match the
    scores-DoubleRow layout: chunk c=(hslot,dj), partition p ->
    col = 64*(p//32 + 4*hslot) + 32*dj + p%32."""
    perm = np.empty(DS, np.int64)
    for c in range(4):
        hslot, dj = c // 2, c % 2
        for p in range(128):
            perm[c * 128 + p] = 64 * (p // 32 + 4 * hslot) + 32 * dj + p % 32
    return perm


def _kernel_body(nc, tc, aps):
    x1, x2f, x2b, wq8, wk8, wv, wo, bq, bk, bv, out = aps

    with ExitStack() as ctx:
        pers = ctx.enter_context(tc.tile_pool(name="pers", bufs=1))

        wq8_sb = pers.tile([64, 2, 2, NKC, 256], FP8)  # [p, half, j, kc, ci]
        wk8_sb = pers.tile([64, 2, 2, NKC, 256], FP8)
        wv_sb = pers.tile([128, NKC, DS], BF16)
        wo_sb = pers.tile([128, 4, D_MODEL], BF16)
        x2f_sb = pers.tile([64, 2, NKC, S], FP8)  # [p, j, kc, t]
        x2b_sb = pers.tile([128, NKC, S], BF16)
        qt8 = pers.tile([128, 2, 2, S], FP8)  # [p, hslot, dj, q]
        kt8 = pers.tile([128, 2, 2, S], FP8)
        v_sb = pers.tile([128, NSC, HPC, DH], BF16)
        aot = pers.tile([128, 4, S], BF16)
        bq_sb = pers.tile([128, 4], F32)
        bk_sb = pers.tile([128, 4], F32)
        bv_bc = pers.tile([128, DS], F32)
        ones = pers.tile([128, 1], BF16)

        px1 = ctx.enter_context(tc.tile_pool(name="px1", bufs=2))
        x1ts = {}

        nc.gpsimd.memset(ones[:, :], 1.0)
        nc.gpsimd.dma_start(
            out=bq_sb, in_=bq.rearrange("(c p) o -> p (c o)", p=128)
        )
        nc.gpsimd.dma_start(
            out=bk_sb, in_=bk.rearrange("(c p) o -> p (c o)", p=128)
        )
        nc.gpsimd.dma_start(
            out=bv_bc, in_=bv.rearrange("s o -> o s").to_broadcast([128, DS])
        )

        # ---- engine-clock model (for exp/quantize/drain placement) ----
        eng_clk = {"a": 0.0, "d": 0.0}
        C_EA, C_ED = 1040.0, 1290.0  # exp per tick
        C_QA, C_QD = 600.0, 670.0  # qk quantize [128,512]
        C_VB = 670.0  # v bias (DVE only)
        C_NRM = 880.0  # normalize per pair (DVE only)
        C_DA, C_DD = 600.0, 670.0  # oproj drain half
        dve_exp_used = [0]

        def pick_eng(ca, cd):
            if eng_clk["a"] + ca <= eng_clk["d"] + cd:
                eng_clk["a"] += ca
                return "a"
            eng_clk["d"] += cd
            return "d"

        def ld_x1(sq, eng):
            t = px1.tile([64, 2, NKC, QB], FP8, tag="x8", name=f"x1_{sq}")
            eng.dma_start(
                out=t,
                in_=x1[:, :, sq * QB : (sq + 1) * QB].rearrange(
                    "p (j c) s -> p j c s", j=2
                ),
            )
            x1ts[sq] = t

        def ld_x2f(sq, eng):
            eng.dma_start(
                out=x2f_sb[:, :, :, sq * QB : (sq + 1) * QB],
                in_=x2f[:, :, sq * QB : (sq + 1) * QB].rearrange(
                    "p (j c) s -> p j c s", j=2
                ),
            )

        def ld_x2b(sq, eng):
            eng.dma_start(
                out=x2b_sb[:, :, sq * QB : (sq + 1) * QB],
                in_=x2b.rearrange("(c p) s -> p c s", p=128)[
                    :, :, sq * QB : (sq + 1) * QB
                ],
            )

        # fp8 DoubleRow projection chunk c of quarter sq -> fp8 qt8/kt8
        # (columns host-permuted so psum partitions land in the
        # scores-DoubleRow layout); bias-add+fp8-quantize on the less
        # loaded of ACT/DVE.
        def qk_chunk(psp, which, sq, c):
            w_sb, dst, b_sb = (
                (wq8_sb, qt8, bq_sb) if which == 1 else (wk8_sb, kt8, bk_sb)
            )
            xt = x1ts[sq] if which == 1 else x2f_sb
            h, c2 = c // 2, c % 2
            ps = psp.tile([128, QB], F32, tag="pp", name="qkp")
            for kc in range(NKC):
                if which == 1:
                    mov = xt[:, :, kc, :]
                else:
                    mov = xt[:, :, kc, sq * QB : (sq + 1) * QB]
                nc.tensor.matmul(
                    ps,
                    w_sb[:, h, :, kc, c2 * 128 : (c2 + 1) * 128],
                    mov,
                    start=(kc == 0),
                    stop=(kc == NKC - 1),
                    perf_mode=DRow,
                )
            dsts = dst[:, c // 2, c % 2, sq * QB : (sq + 1) * QB]
            if pick_eng(C_QA, C_QD) == "a":
                nc.scalar.activation(dsts, ps, Ident, bias=b_sb[:, c : c + 1])
            else:
                nc.vector.tensor_scalar_add(dsts, ps, b_sb[:, c : c + 1])

        def v_chunk(psp, sq, sc):
            ps = psp.tile([128, QB], F32, tag="pp", name="vp")
            for kc in range(NKC):
                nc.tensor.matmul(
                    ps,
                    x2b_sb[:, kc, sq * QB + sc * 128 : sq * QB + (sc + 1) * 128],
                    wv_sb[:, kc, :],
                    start=(kc == 0),
                    stop=(kc == NKC - 1),
                )
            eng_clk["d"] += C_VB
            nc.vector.tensor_add(
                v_sb[:, sq * 4 + sc, :, :],
                ps.rearrange("p (h d) -> p h d", h=HPC),
                bv_bc.rearrange("p (h d) -> p h d", h=HPC),
            )

        # ---- load streams ----
        # DMA transfers serialize on the modeled (exclusive) DMA device, so
        # order IS the schedule. Scalar carries only the loads the first
        # scores need; everything else on sync. ACT/DVE never issue DMAs
        # (their 667ns seq dispatch would stall the exp stream).
        dma_ready = {}
        _dma_clk = [2000.0]

        def dma_ns(total_bytes, elem):
            lat = 2.0 if elem < 512 else 1.0
            return total_bytes / elem / 16.0 * max(elem * lat / 22.5, 7.0)

        def _track(name, total_bytes, elem, emit):
            emit()
            _dma_clk[0] += dma_ns(total_bytes, elem) + 200.0
            dma_ready[name] = _dma_clk[0]

        KB = 1024
        _track("wk8h", 256 * KB, 4096,
               lambda: nc.scalar.dma_start(out=wk8_sb[:, 0], in_=wk8[:, 0, :].rearrange("p (j c i) -> p j c i", j=2, c=NKC)))
        _track("x2f0", 512 * KB, 512, lambda: ld_x2f(0, nc.scalar))
        _track("wq8h", 256 * KB, 4096,
               lambda: nc.sync.dma_start(out=wq8_sb[:, 0], in_=wq8[:, 0, :].rearrange("p (j c i) -> p j c i", j=2, c=NKC)))
        _track("x1q0", 512 * KB, 512, lambda: ld_x1(0, nc.sync))
        _track("x2f1", 512 * KB, 512, lambda: ld_x2f(1, nc.sync))
        _track("x2f2", 512 * KB, 512, lambda: ld_x2f(2, nc.sync))
        _track("x2b0", 1024 * KB, 1024, lambda: ld_x2b(0, nc.sync))
        _track("x2f3", 512 * KB, 512, lambda: ld_x2f(3, nc.sync))
        _track("wv", 1024 * KB, 1024,
               lambda: nc.sync.dma_start(out=wv_sb, in_=wv.rearrange("(c p) o -> p c o", p=128)))
        _track("x2b1", 1024 * KB, 1024, lambda: ld_x2b(1, nc.sync))
        _track("x2b2", 1024 * KB, 1024, lambda: ld_x2b(2, nc.sync))

        # startup projection chunks in their own psum pool. Dummy matmuls
        # first ramp the PE clock out of its cold p-state.
        warm = pers.tile([128, 512], BF16)
        nc.gpsimd.memset(warm[:, :], 0.0)
        with tc.tile_pool(name="psA", bufs=4, space="PSUM") as psA:
            wps = psA.tile([128, 512], F32, tag="pp", name="warmp")
            for i in range(8):
                nc.tensor.matmul(
                    wps[0:1, :],
                    ones[:, 0:1],
                    warm[:, :],
                    start=(i == 0),
                    stop=(i == 7),
                )
            qk_chunk(psA, 1, 0, 0)
            qk_chunk(psA, 2, 0, 0)
            qk_chunk(psA, 1, 0, 1)
            qk_chunk(psA, 2, 0, 1)

        # ---- attention pools ----
        attn_ctx = ctx.enter_context(ExitStack())
        psc = attn_ctx.enter_context(tc.tile_pool(name="psc", bufs=2, space="PSUM"))
        pso = attn_ctx.enter_context(tc.tile_pool(name="pso", bufs=2, space="PSUM"))
        pdn = attn_ctx.enter_context(tc.tile_pool(name="pdn", bufs=1, space="PSUM"))
        psp = attn_ctx.enter_context(tc.tile_pool(name="psp", bufs=1, space="PSUM"))
        pex = attn_ctx.enter_context(tc.tile_pool(name="pex", bufs=EX_BUFS))
        pao = attn_ctx.enter_context(tc.tile_pool(name="pao", bufs=2))
        prd = attn_ctx.enter_context(tc.tile_pool(name="prd", bufs=2))
        pot = attn_ctx.enter_context(tc.tile_pool(name="pot", bufs=2))

        ot_tiles = {}

        def oproj_half(mb, nt):
            if nt == 0:
                ot_tiles[mb] = pot.tile(
                    [128, D_MODEL], BF16, tag="ot", name="oti"
                )
            ot = ot_tiles[mb]
            ps = psp.tile([128, 512], F32, tag="pp", name="opp")
            for kc in range(4):
                nc.tensor.matmul(
                    ps,
                    aot[:, kc, mb * 128 : (mb + 1) * 128],
                    wo_sb[:, kc, nt * 512 : (nt + 1) * 512],
                    start=(kc == 0),
                    stop=(kc == 3),
                )
            if pick_eng(C_DA, C_DD) == "a":
                nc.scalar.activation(ot[:, nt * 512 : (nt + 1) * 512], ps, Copy)
            else:
                nc.vector.tensor_copy(ot[:, nt * 512 : (nt + 1) * 512], ps)
            if nt == 1:
                nc.gpsimd.dma_start(
                    out=out[mb * 128 : (mb + 1) * 128, :], in_=ot
                )

        # ---- filler schedule: tick -> thunks ----
        # chunk c=(hslot,dj) of a quarter serves pairs 2*(c//2), 2*(c//2)+1.
        CQK, CV, COPH = 860, 1707, 860
        T0, TICK = 9000.0, 680.0

        def r2t(ns):
            return max(0, int((ns - T0) / TICK) + 1)

        # jobs: (deadline_tick, ready_tick, pe_cost, thunk, tag). One heavy
        # job per tick, earliest-deadline-first among ready jobs. qt/kt
        # chunks MUST land by their deadline (scores would otherwise read
        # uninitialized sbuf = a real race); v jobs may slip (attn@V release
        # is gated on the v chunk being emitted, the backlog absorbs it).
        jobs = []
        for s in range(1, 4):
            for c in range(2):
                jobs.append(
                    (4 * s - 3 + c, r2t(dma_ready[f"x2f{s}"]), CQK,
                     lambda s=s, c=c: qk_chunk(psp, 2, s, c), None)
                )
        # pair 2 (tick 32+4s) reads BOTH dj chunks (c2 and c3) of hslot 1
        # c2/c3 need the half-1 weight loads dispatched at fill ticks 3/4:
        # ready >= 6 also orders the emission after those dma_starts.
        for s in range(4):
            rd = r2t(dma_ready[f"x2f{s}"])
            jobs.append(
                (30 + 4 * s, max(rd, 6), CQK,
                 lambda s=s: qk_chunk(psp, 2, s, 2), None)
            )
            jobs.append(
                (31 + 4 * s, max(rd, 6), CQK,
                 lambda s=s: qk_chunk(psp, 2, s, 3), None)
            )
        for c in (2, 3):
            jobs.append(
                (28 + c, max(r2t(dma_ready["x1q0"]), 6), CQK,
                 lambda c=c: qk_chunk(psp, 1, 0, c), None)
            )
        for s in range(4):
            rv = r2t(max(dma_ready.get(f"x2b{s}", 0.0), dma_ready["wv"])) \
                if s < 3 else 28
            for sc in range(4):
                dl = max(4 * s + sc + AGE - 1, rv)
                jobs.append(
                    (dl, rv, CV,
                     lambda s=s, sc=sc: v_chunk(psp, s, sc), ("v", 4 * s + sc))
                )
        for sq in range(1, 4):
            for c in range(4):
                jobs.append(
                    (64 * sq - 3 + c if c < 2 else 64 * sq + 30 + c,
                     64 * (sq - 1) + 6, CQK,
                     lambda sq=sq, c=c: qk_chunk(psp, 1, sq, c), None)
                )
        # oproj halves: the source qb's last transpose is emitted when its
        # final attn@V leaves the backlog (qb end + AGE)
        for mb in range(12):
            for nt in range(2):
                jobs.append(
                    (1000 + 2 * mb + nt, 64 * (mb // 4 + 1) + AGE + 2, COPH,
                     lambda mb=mb, nt=nt: oproj_half(mb, nt), None)
                )
        jobs.sort(key=lambda j: (j[0], j[1]))
        # Precompute the EDF tick assignment; qt/kt jobs must land by their
        # deadline, v jobs gate attn@V release below.
        v_done_tick = {}
        _pending = list(range(len(jobs)))
        assigned = {}
        for t in range(256):
            pick = None
            for idx in _pending:
                if jobs[idx][0] <= t or jobs[idx][1] <= t:
                    pick = idx
                    break
            if pick is not None:
                assigned[t] = pick
                _pending.remove(pick)
                tag = jobs[pick][4]
                if tag and tag[0] == "v":
                    v_done_tick[tag[1]] = t
        assert not _pending, f"{len(_pending)} jobs unassigned"
        assert len(v_done_tick) == 16
        # zero-cost emissions (loads) at fixed ticks
        fill = defaultdict(list)
        fill[2].append(lambda: ld_x2b(3, nc.sync))
        fill[3].append(
            lambda: nc.sync.dma_start(out=wk8_sb[:, 1], in_=wk8[:, 1, :].rearrange("p (j c i) -> p j c i", j=2, c=NKC))
        )
        fill[4].append(
            lambda: nc.sync.dma_start(out=wq8_sb[:, 1], in_=wq8[:, 1, :].rearrange("p (j c i) -> p j c i", j=2, c=NKC))
        )
        fill[5].append(lambda: ld_x1(1, nc.sync))
        for sq in range(2, 4):
            fill[64 * (sq - 1) + 2].append(lambda sq=sq: ld_x1(sq, nc.sync))
        fill[20].append(
            lambda: nc.sync.dma_start(
                out=wo_sb, in_=wo.rearrange("(c p) o -> p c o", p=128)
            )
        )

        # ---- flat pipelined attention stream ----
        TICKS = [
            (qb, pair, kc)
            for qb in range(4)
            for pair in range(4)
            for kc in range(NSC)
        ]
        sc_tiles = {}
        ex_tiles = {}
        state = {}

        def emit_sc(t):
            qb, pair, kc = TICKS[t]
            q0 = qb * QB
            scp = psc.tile([128, 2, QB], F32, tag="sc", name="scp")
            for par in range(2):
                h = 2 * pair + par
                hb = 32 * (h % 4)
                nc.tensor.matmul(
                    scp[:, par, :],
                    kt8[hb : hb + 32, h // 4, :, kc * 128 : (kc + 1) * 128],
                    qt8[hb : hb + 32, h // 4, :, q0 : q0 + QB],
                    start=True,
                    stop=True,
                    perf_mode=DRow,
                    tile_position=(hb, 0),
                )
            sc_tiles[t] = scp

        def emit_exp(t, ready_ns):
            expt = pex.tile([128, 2, QB], BF16, tag="ex", name="ext")
            scp = sc_tiles.pop(t)
            fa = max(eng_clk["a"], ready_ns) + C_EA
            fd = max(eng_clk["d"], ready_ns) + C_ED
            if fd < fa and dve_exp_used[0] < DVE_EXP_MAX:
                dve_exp_used[0] += 1
                eng_clk["d"] = fd
                nc.vector.tensor_scalar(
                    expt[:, :, :].bitcast(I16), scp, AEXP, BEXP, Mult, Add
                )
            else:
                eng_clk["a"] = fa
                nc.scalar.activation(expt, scp, Exp, scale=float(SCALE))
            ex_tiles[t] = expt

        def emit_av(t):
            qb, pair, kc = TICKS[t]
            expt = ex_tiles.pop(t)
            if kc == 0:
                state[("outp", pair % 2)] = pso.tile(
                    [128, 2, 4, DH], F32, tag="acc", name="outp"
                )
                if pair == 0:
                    state["den"] = pdn.tile([128, 32], F32, tag="dn", name="den")
                    state["rden"] = prd.tile([128, 32], F32, tag="rd", name="rden")
            outp = state[("outp", pair % 2)]
            den = state["den"]
            for par in range(2):
                h = 2 * pair + par
                for qc in range(4):
                    exs = expt[:, par, qc * 128 : (qc + 1) * 128]
                    first = kc == 0 and par == 0 and qc == 0
                    last = kc == NSC - 1 and par == 1 and qc == 3
                    nc.tensor.matmul(
                        outp[:, par, qc, :],
                        exs,
                        v_sb[:, kc, h, :],
                        start=first,
                        stop=last,
                    )
                    di = pair * 8 + par * 4 + qc
                    nc.tensor.matmul(
                        den[:, di : di + 1],
                        exs,
                        ones[:, :],
                        start=first,
                        stop=last,
                    )
            if kc == NSC - 1:
                q0 = qb * QB
                rden = state["rden"]
                rsl = rden[:, pair * 8 : (pair + 1) * 8]
                nc.vector.reciprocal(rsl, den[:, pair * 8 : (pair + 1) * 8])
                ao = pao.tile([128, 4, 2, DH], BF16, tag="ao", name="ao")
                nc.vector.tensor_tensor(
                    out=ao[:, :, :, :],
                    in0=outp.rearrange("p a b d -> p b a d"),
                    in1=rsl.rearrange("p (a b) -> p b a", a=2).to_broadcast(
                        [128, 4, 2, DH]
                    ),
                    op=Mult,
                )
                eng_clk["d"] += C_NRM
                for qc in range(4):
                    nc.sync.dma_start_transpose(
                        aot[:, pair, q0 + qc * 128 : q0 + (qc + 1) * 128],
                        ao[:, qc, :, :],
                    )

        # Greedy emission: track modeled PE/ACT/DVE clocks; defer attn@V
        # work (bounded backlog) and drain it age-based so the ex pool never
        # starves the exp stream. Never release an attn@V whose v chunk
        # hasn't been emitted yet (emission order defines dependency order).
        C_SC, C_AV = 213.0, 220.0
        pe_t = 5500.0  # first matmul lands after the startup DMA chain
        eng_clk["a"] = eng_clk["d"] = 7000.0
        sc_done = {}
        backlog = []

        emit_sc(0)
        sc_done[0] = pe_t = pe_t + C_SC
        emit_sc(1)
        sc_done[1] = pe_t = pe_t + C_SC
        for t in range(256):
            emit_exp(t, sc_done[t] + 100.0)
            backlog.append(t)
            for f in fill[t]:
                f()
            if t in assigned:
                dl, rd, cost, th, tag = jobs[assigned[t]]
                th()
                pe_t += cost
            # attn@V after the job: these small matmuls overlap the job's
            # psum-drain latency so back-to-back fills don't bubble PE.
            while backlog and (
                len(backlog) > MAX_BACKLOG
                or (len(backlog) > D_MIN and t - backlog[0] >= AGE)
            ):
                qbu, pairu, kcu = TICKS[backlog[0]]
                if qbu == 0 and v_done_tick[kcu] > t:
                    break
                emit_av(backlog.pop(0))
                pe_t += C_AV
            # exp-gated score matmul last, so jobs/attn@V never sit behind
            # the gate in the PE queue
            if t + 2 < 256:
                emit_sc(t + 2)
                pe_t += C_SC
                sc_done[t + 2] = pe_t
        for u in backlog:
            emit_av(u)

        attn_ctx.close()

        # ---- tail: last output-projection blocks ----
        with tc.tile_pool(name="psD", bufs=4, space="PSUM") as psD, tc.tile_pool(
            name="potD", bufs=2
        ) as potD:
            for mb in range(12, 16):
                ot = potD.tile([128, D_MODEL], BF16, tag="ot", name="otd")
                for nt in range(2):
                    ps = psD.tile([128, 512], F32, tag="pf", name="opd")
                    for kc in range(4):
                        nc.tensor.matmul(
                            ps,
                            aot[:, kc, mb * 128 : (mb + 1) * 128],
                            wo_sb[:, kc, nt * 512 : (nt + 1) * 512],
                            start=(kc == 0),
                            stop=(kc == 3),
                        )
                    if nt == 0:
                        nc.scalar.activation(ot[:, 0:512], ps, Copy)
                    else:
                        nc.vector.tensor_copy(ot[:, 512:1024], ps)
                nc.gpsimd.dma_start(out=out[mb * 128 : (mb + 1) * 128, :], in_=ot)


_NC_CACHE = []


def _build():
    if _NC_CACHE:
        return _NC_CACHE[0]
    nc = bacc.Bacc(None, target_bir_lowering=False, debug=False)
    x1 = nc.dram_tensor("x1", [64, 16, S], FP8, kind="ExternalInput")
    x2f = nc.dram_tensor("x2f", [64, 16, S], FP8, kind="ExternalInput")
    x2b = nc.dram_tensor("x2b", [D_IN, S], BF16, kind="ExternalInput")
    wq8 = nc.dram_tensor("wq8", [64, 2, 4096], FP8, kind="ExternalInput")
    wk8 = nc.dram_tensor("wk8", [64, 2, 4096], FP8, kind="ExternalInput")
    wv = nc.dram_tensor("wv", [D_IN, DS], BF16, kind="ExternalInput")
    wo = nc.dram_tensor("wo", [DS, D_MODEL], BF16, kind="ExternalInput")
    bq = nc.dram_tensor("bq", [DS, 1], F32, kind="ExternalInput")
    bk = nc.dram_tensor("bk", [DS, 1], F32, kind="ExternalInput")
    bv = nc.dram_tensor("bv", [DS, 1], F32, kind="ExternalInput")
    out = nc.dram_tensor("out", [S, D_MODEL], BF16, kind="ExternalOutput")
    with tile.TileContext(nc) as tc:
        _kernel_body(
            nc,
            tc,
            aps=(
                x1[:, :, :],
                x2f[:, :, :],
                x2b[:, :],
                wq8[:, :, :],
                wk8[:, :, :],
                wv[:, :],
                wo[:, :],
                bq[:, :],
                bk[:, :],
                bv[:, :],
                out[:, :],
            ),
        )
    nc.compile()
    _NC_CACHE.append(nc)
    return nc


def _run(inputs, trace=False, **kw):
    import ml_dtypes

    nc = _build()
    F8 = ml_dtypes.float8_e4m3
    BF = ml_dtypes.bfloat16
    f32 = lambda a: np.ascontiguousarray(np.asarray(a, dtype=np.float32))
    perm = _col_perm()
    X1, X2 = (
        np.asarray(inputs["X1"], np.float32),
        np.asarray(inputs["X2"], np.float32),
    )
    Wq, Wk = np.asarray(inputs["Wq"], np.float32), np.asarray(
        inputs["Wk"], np.float32
    )
    Wv, Wo = np.asarray(inputs["Wv"], np.float32), np.asarray(
        inputs["Wo"], np.float32
    )
    bqf, bkf = (
        np.asarray(inputs["bq"], np.float32),
        np.asarray(inputs["bk"], np.float32),
    )

    def to_x8(Xb):  # [S, D_IN] -> [64, 16, S] fp8, d = 128*kc + 64*j + p
        a = Xb.T.reshape(NKC, 2, 64, S).transpose(2, 1, 0, 3)
        return np.ascontiguousarray(a.reshape(64, 16, S)).astype(F8)

    def to_w8(Ws):  # [D_IN, DS] (col-perm'd) -> [64, 2, 4096]
        a = Ws.reshape(NKC, 2, 64, 2, 256).transpose(2, 3, 1, 0, 4)
        return np.ascontiguousarray(a.reshape(64, 2, 4096)).astype(F8)

    in_maps = []
    for c in range(8):
        b, hf = c // 2, c % 2
        sl = slice(hf * DS, (hf + 1) * DS)
        wq_s, wk_s = Wq[:, sl][:, perm], Wk[:, sl][:, perm]
        in_maps.append(
            {
                "x1": to_x8(X1[b]),
                "x2f": to_x8(X2[b]),
                "x2b": np.ascontiguousarray(X2[b].T).astype(BF),
                "wq8": to_w8(wq_s),
                "wk8": to_w8(wk_s),
                "wv": np.ascontiguousarray(Wv[:, sl]).astype(BF),
                "wo": np.ascontiguousarray(Wo[sl, :]).astype(BF),
                "bq": np.ascontiguousarray(bqf[sl][perm]).reshape(DS, 1),
                "bk": np.ascontiguousarray(bkf[sl][perm]).reshape(DS, 1),
                "bv": f32(inputs["bv"])[sl].reshape(DS, 1),
            }
        )
    res = run_bass_kernel_spmd(nc, in_maps, list(range(8)), trace=trace, **kw)
    parts = [np.asarray(res.results[c]["out"], np.float32) for c in range(8)]
    bo = f32(inputs["bo"])
    full = np.stack(
        [parts[2 * b] + parts[2 * b + 1] + bo[None, :] for b in range(B)]
    )
    return full.astype(np.float32), res


def kernel(**inputs):
    out, _ = _run(inputs, trace=False)
    return out
